# revision 15
# baseline (speedup 1.0000x reference)
"""Trainium2 Bass kernel for GPUTimeMask: zero out per-batch time windows.

Semantics (matches reference):
    out = x.copy();  for m, b:  out[b, :, s[m,b] : s[m,b]+clip(w[m,b],1,150)] = 0

Strategy (v3 — DRAM->DRAM streaming on three queues):
  - Shard x along the CHANNEL axis: 16 channels -> 2 per core across 8 cores.
    Every core holds ALL 64 batch rows, so the (runtime-valued) mask windows
    live at identical local coordinates on every core -> one SPMD program
    with window offsets specialized in at build time.
  - Per core the output is a byte-for-byte copy of the input except ~128
    tiny windows (<= 2 rows x 150 cols).  Instead of staging through SBUF
    (which pins the kernel to the ~435 GB/s SBUF-AXI fabric ceiling), issue
    big DRAM->DRAM DMA copies: each SDMA descriptor reads and writes HBM
    inline, so HBM runs duplex (~650 GB/s measured -> ~330 GB/s copy rate,
    the 16 SDMA engines' read->write turnaround) with no SBUF pipeline, no
    buffer-reuse WARs, and no compute engines in the path.  D2D moves each
    byte with ONE descriptor instead of two, so it is ~1.3x more
    SDMA-engine-efficient than any SBUF-staged scheme.
  - The plane [128, 60000] f32 is split into 32 contiguous 4-row groups
    (960 KB each -> 32 descriptors of 30 KB) round-robined over THREE
    issue queues: qSP + qAct (HWDGE) and the gpsimd SWDGE queue.
  - Mask windows are overwritten with zeros by tiny DMAs sourced from a
    memset SBUF tile.  Each queue fills the windows of its OWN groups,
    interleaved behind its copy stream with a 3-group lag: by the time the
    sequencer reaches "wait for group g's copy, then fill g's windows",
    that copy has (nearly) completed, so the waits barely stall and no
    queue builds a fill backlog.  Crucially the tiny fill descriptors sit
    BETWEEN big copy descriptors in each ring: tiny packets process at
    only ~4/us when bunched or on a dedicated queue (their HBM round-trip
    latency doesn't pipeline), but hide behind big transfers when spread.
  - Raw bass (no TileContext): semaphores placed by hand, one wait per
    instruction, and the only end-of-kernel cost is the terminal waits.
  - Programs are cached keyed on (starts, widths) bytes.
"""

import sys

import numpy as np

for _p in ("/opt/trn_rl_repo",):
    if _p not in sys.path:
        sys.path.insert(0, _p)

import concourse.bass as bass
import concourse.mybir as mybir
from concourse.bass_utils import run_bass_kernel_spmd

B, C, T = 64, 16, 60000
MAX_MASK_WIDTH = 150
N_CORES = 8
C_LOCAL = C // N_CORES          # 2 channels per core
P = B * C_LOCAL                 # 128 rows: row = b * C_LOCAL + c_local

GROUP_ROWS = 4                  # 2 batches; contiguous 960 KB per group
N_GROUPS = P // GROUP_ROWS      # 32
N_QUEUES = 3                    # qSP, qAct (HWDGE) + gpsimd (SWDGE)
PRIME = 3                       # copies enqueued ahead of the first fill wait

_program_cache: dict[bytes, bass.Bass] = {}


def _merged_windows(starts: np.ndarray, widths: np.ndarray) -> list[list[tuple[int, int]]]:
    """Per-batch union of mask intervals (merge overlapping/adjacent)."""
    w = np.clip(widths, 1, MAX_MASK_WIDTH)
    out: list[list[tuple[int, int]]] = []
    for b in range(B):
        ivs = sorted(
            (int(starts[m, b]), min(int(starts[m, b]) + int(w[m, b]), T))
            for m in range(starts.shape[0])
        )
        merged = [ivs[0]]
        for s, e in ivs[1:]:
            if s <= merged[-1][1]:
                merged[-1] = (merged[-1][0], max(merged[-1][1], e))
            else:
                merged.append((s, e))
        out.append([(s, e) for s, e in merged if s < e])
    return out


def _build_program(windows: list[list[tuple[int, int]]]) -> bass.Bass:
    """windows[b]: merged (lo, hi) column ranges to zero; identical per core."""
    nc = bass.Bass()
    x = nc.declare_dram_parameter("x", [P, T], mybir.dt.float32, isOutput=False)
    y = nc.declare_dram_parameter("y", [P, T], mybir.dt.float32, isOutput=True)

    copy_sems = [nc.alloc_semaphore(f"copy_g{g}") for g in range(N_GROUPS)]
    fill_sems = [nc.alloc_semaphore(f"fills_q{q}") for q in range(N_QUEUES)]
    engines = [nc.sync, nc.scalar, nc.gpsimd]

    def group_fills(g):
        out = []
        b0 = g * GROUP_ROWS // C_LOCAL
        for b in range(b0, b0 + GROUP_ROWS // C_LOCAL):
            for lo, hi in windows[b]:
                out.append((b * C_LOCAL, lo, hi))
        return out

    with nc.sbuf_tensor("zeros", [32, MAX_MASK_WIDTH + 2], mybir.dt.float32) as zsb:
        qgroups = [[g for g in range(N_GROUPS) if g % N_QUEUES == q] for q in range(N_QUEUES)]
        n_fills_q = [0] * N_QUEUES
        # gpsimd's ~0.84 us/DMA fill issue would trail ~4 us past the final
        # copy for its LAST group; those fills go to the HWDGE stream ends.
        handoff = qgroups[2][-1]
        hb = handoff * GROUP_ROWS // C_LOCAL  # first batch of that group

        for q, eng in enumerate(engines):
            gs = qgroups[q]

            def copy(g):
                r0, r1 = g * GROUP_ROWS, (g + 1) * GROUP_ROWS
                eng.dma_start(
                    out=y[r0:r1, :], in_=x[r0:r1, :], max_dma_last_dim=30720
                ).then_inc(copy_sems[g], 16)

            def fill_windows(fw):
                for row, lo, hi in fw:
                    eng.dma_start(
                        out=y[row : row + C_LOCAL, lo:hi],
                        in_=zsb[0:C_LOCAL, 0 : hi - lo],
                    ).then_inc(fill_sems[q], 16)
                    n_fills_q[q] += 1

            # copies first: the zeros handshake gates only the fills
            for g in gs[:PRIME]:
                copy(g)
            if q == 2:
                nc.gpsimd.memset(zsb[:], 0.0)
                for qq in range(N_QUEUES):
                    nc.gpsimd.sem_inc(fill_sems[qq], 1)
            else:
                eng.wait_ge(fill_sems[q], 1)
            for i, g in enumerate(gs):
                if i + PRIME < len(gs):
                    copy(gs[i + PRIME])
                if not (q == 2 and g == handoff):
                    eng.wait_ge(copy_sems[g], 16)
                    fill_windows(group_fills(g))

        # handed-off fills: one batch per HWDGE queue, after their own loops
        for q, eng in ((0, nc.sync), (1, nc.scalar)):
            b = hb + q
            eng.wait_ge(copy_sems[handoff], 16)
            for lo, hi in windows[b]:
                eng.dma_start(
                    out=y[b * C_LOCAL : b * C_LOCAL + C_LOCAL, lo:hi],
                    in_=zsb[0:C_LOCAL, 0 : hi - lo],
                ).then_inc(fill_sems[q], 16)
                n_fills_q[q] += 1

        for q, eng in enumerate(engines):
            for g in qgroups[q]:
                eng.wait_ge(copy_sems[g], 16)
            eng.wait_ge(fill_sems[q], 16 * n_fills_q[q] + 1)

    return nc


def _get_program(starts: np.ndarray, widths: np.ndarray) -> bass.Bass:
    key = starts.tobytes() + widths.tobytes()
    prog = _program_cache.get(key)
    if prog is None:
        prog = _build_program(_merged_windows(starts, widths))
        _program_cache[key] = prog
    return prog


def _run(x, starts, widths, trace=False, tmpdir=None):
    x = np.ascontiguousarray(x, dtype=np.float32)
    starts = np.asarray(starts, dtype=np.int32)
    widths = np.asarray(widths, dtype=np.int32)
    assert x.shape == (B, C, T), x.shape

    nc = _get_program(starts, widths)
    in_maps = [
        {
            "x": np.ascontiguousarray(
                x[:, k * C_LOCAL : (k + 1) * C_LOCAL, :]
            ).reshape(P, T)
        }
        for k in range(N_CORES)
    ]
    res = run_bass_kernel_spmd(
        nc, in_maps, list(range(N_CORES)), trace=trace, tmpdir=tmpdir
    )

    out = np.empty_like(x)
    for k in range(N_CORES):
        out[:, k * C_LOCAL : (k + 1) * C_LOCAL, :] = res.results[k]["y"].reshape(
            B, C_LOCAL, T
        )
    return out, res


def kernel(x, starts, widths):
    out, _ = _run(x, starts, widths, trace=False)
    return out
